# revision 1
# baseline (speedup 1.0000x reference)
"""LoRA QKV projection kernel for Trainium2 (Bass/Tile), 8-core SPMD.

Problem: x [B=4, S=2048, D=4096] fp32; for each of q/k/v:
    out = x @ W.T + (x @ A.T) @ B.T      (W [H=4096, D], A [R=16, D], B [H, R])

Sharding: data-parallel over tokens. Each of the 8 cores owns 1024 of the
8192 tokens and computes all 3*4096 output columns for them. Weights are
replicated. Host-side prep is layout-only (transpose/slice/stack) so that
the contraction dim D lands on SBUF partitions on-chip.

On-device math runs the tensor engine in float32r mode (fp32 storage,
reduced-precision multiply): measured ~233 ns per 128x512 matmul (same as
bf16, 4x faster than fp32) at ~1.5e-4 max rel err vs fp64.
"""

import sys
import types

import numpy as np

import concourse.bass as bass
import concourse.mybir as mybir
import concourse.tile as tile
from concourse import bacc, bass_utils


def _install_profiling_shim():
    """Make trace=True usable under axon on images whose ``antenv`` lacks
    ``axon_hooks``: inject the module and register the ctypes NTFF hook.
    Harmless no-op when the real module exists. Also keep profile artifacts
    local (no bucket upload is available here)."""
    try:
        if "antenv.axon_hooks" not in sys.modules:
            try:
                from antenv import axon_hooks  # noqa: F401
            except ImportError:
                mod = types.ModuleType("antenv.axon_hooks")
                mod._hook = None
                mod.set_axon_ntff_profile_hook = lambda h: setattr(
                    mod, "_hook", h)
                mod.get_axon_ntff_profile_hook = lambda: mod._hook
                sys.modules["antenv.axon_hooks"] = mod
                import antenv
                antenv.axon_hooks = mod
                try:
                    from trn_agent_boot.trn_boot import _ntff_profile_via_ctypes
                    hook = _ntff_profile_via_ctypes("/opt/axon/libaxon_pjrt.so")
                    if hook is not None:
                        mod.set_axon_ntff_profile_hook(hook)
                except Exception:
                    pass
        bass_utils.upload_artifacts = lambda tmpdir: "local://" + str(tmpdir)
    except Exception:
        pass


_install_profiling_shim()

F32 = mybir.dt.float32
F32R = mybir.dt.float32r

N_CORES = 8
P = 128          # partition dim
NCH = 512        # matmul moving free dim / psum bank width (fp32)


def _build(D, T, H, n_cores=N_CORES):
    """Build the per-core Bass program.

    D: model dim (contraction), T: tokens per core, H: output columns per
    projection. All multiples of the tile sizes used below.
    """
    DT = D // P           # d-tiles
    ST = T // P           # token tiles per core (psum accumulators)
    CH_PER_PROJ = H // NCH
    NCHUNK = 3 * CH_PER_PROJ  # h-chunks across q,k,v

    assert ST <= 8, "token tiles must fit in the 8 psum banks"

    nc = bacc.Bacc("TRN2", target_bir_lowering=False, debug=False,
                   num_devices=n_cores)

    xT_d = nc.dram_tensor("xT", [D, T], F32, kind="ExternalInput")
    wT_d = nc.dram_tensor("wT", [D, 3 * H], F32, kind="ExternalInput")
    aT_d = nc.dram_tensor("aT", [D, 48], F32, kind="ExternalInput")
    bT_d = nc.dram_tensor("bT", [3, 16, H], F32, kind="ExternalInput")
    outs_d = [
        nc.dram_tensor(name, [T, H], F32, kind="ExternalOutput")
        for name in ("q", "k", "v")
    ]

    with tile.TileContext(nc) as tc:
        with (
            tc.tile_pool(name="stage", bufs=3) as stage,
            tc.tile_pool(name="xtr", bufs=DT) as xtr,
            tc.tile_pool(name="wr", bufs=5) as wr,
            tc.tile_pool(name="lora", bufs=1) as lora,
            tc.tile_pool(name="lorab", bufs=2) as lorab,
            tc.tile_pool(name="psum", bufs=8, space="PSUM") as psum,
            tc.tile_pool(name="outsb", bufs=4) as outsb,
        ):
            # ---- LoRA A tiles first: tiny DMAs must not queue behind the
            # 16 MB x load, or the xa.T prologue can't fill the x window ----
            at_r = []
            for pj in range(3):
                a_st = stage.tile([P, DT, 16], F32, tag="st")
                nc.sync.dma_start(
                    a_st[:],
                    aT_d[:, pj * 16:(pj + 1) * 16].rearrange(
                        "(dt p) r -> p dt r", p=P),
                )
                a_r = lora.tile([P, DT, 16], F32R, tag=f"a{pj}",
                                name=f"a_{pj}")
                nc.vector.tensor_copy(a_r[:], a_st[:])
                at_r.append(a_r)

            # ---- x load: one tile per d-block (fine-grained deps) ----
            xt = [xtr.tile([P, T], F32R, tag="xt", name=f"xt_{d}")
                  for d in range(DT)]
            for d in range(DT):
                st = stage.tile([P, T], F32, tag="st", name=f"xst_{d}")
                nc.sync.dma_start(st[:], xT_d[d * P:(d + 1) * P, :])
                nc.vector.tensor_copy(xt[d][:], st[:])

            # ---- xa.T = (x @ A.T).T per projection: [16, T] f32r.
            # Runs DMA-paced inside the x-load window, warming the PE. ----
            SC = T // NCH if T >= NCH else 1
            SCW = min(T, NCH)
            xat_r = []
            for pj in range(3):
                xa_r = lora.tile([16, T], F32R, tag=f"xa{pj}",
                                 name=f"xa_{pj}")
                for sc in range(SC):
                    ps = psum.tile([16, SCW], F32, tag="ps")
                    for d in range(DT):
                        nc.tensor.matmul(
                            ps[:],
                            at_r[pj][:, d, :],
                            xt[d][:, sc * SCW:(sc + 1) * SCW],
                            start=(d == 0),
                            stop=(d == DT - 1),
                        )
                    nc.vector.tensor_copy(
                        xa_r[:, sc * SCW:(sc + 1) * SCW], ps[:])
                xat_r.append(xa_r)

            # ---- main loop: stream W.T chunks, accumulate in psum banks ----
            for j in range(NCHUNK):
                pj, hoff = j // CH_PER_PROJ, (j % CH_PER_PROJ) * NCH
                ps_tiles = [psum.tile([P, NCH], F32, tag="ps",
                                      name=f"ps_{j}_{s}")
                            for s in range(ST)]
                b_st = stage.tile([16, NCH], F32, tag="st")
                nc.sync.dma_start(b_st[:], bT_d[pj, :, hoff:hoff + NCH])
                b_r = lorab.tile([16, NCH], F32R)
                nc.vector.tensor_copy(b_r[:], b_st[:])
                for d in range(DT):
                    w_st = stage.tile([P, NCH], F32, tag="wst")
                    nc.sync.dma_start(
                        w_st[:],
                        wT_d[d * P:(d + 1) * P,
                             pj * H + hoff:pj * H + hoff + NCH],
                    )
                    w_r = wr.tile([P, NCH], F32R)
                    nc.vector.tensor_copy(w_r[:], w_st[:])
                    for s in range(ST):
                        nc.tensor.matmul(
                            ps_tiles[s],
                            xt[d][:, s * P:(s + 1) * P],
                            w_r[:],
                            start=(d == 0),
                            stop=False,
                        )
                for s in range(ST):
                    # LoRA rank-16 contribution closes the accumulation group
                    nc.tensor.matmul(
                        ps_tiles[s],
                        xat_r[pj][:, s * P:(s + 1) * P],
                        b_r[:],
                        start=False,
                        stop=True,
                    )
                for s in range(ST):
                    ot = outsb.tile([P, NCH], F32)
                    nc.vector.tensor_copy(ot[:], ps_tiles[s])
                    nc.sync.dma_start(
                        outs_d[pj][s * P:(s + 1) * P, hoff:hoff + NCH],
                        ot[:],
                    )

    nc.compile()
    return nc


_NC_CACHE = {}


def _get_nc(D, T, H):
    key = (D, T, H)
    if key not in _NC_CACHE:
        _NC_CACHE[key] = _build(D, T, H)
    return _NC_CACHE[key]


def _run(x, q_weight, k_weight, v_weight, q_A, q_B, k_A, k_B, v_A, v_B,
         trace=False):
    Bb, S, D = x.shape
    H = q_weight.shape[0]
    TOK = Bb * S
    T = TOK // N_CORES

    nc = _get_nc(D, T, H)

    xT = np.ascontiguousarray(
        np.asarray(x, dtype=np.float32).reshape(TOK, D).T)
    wT = np.ascontiguousarray(
        np.concatenate(
            [np.asarray(w, dtype=np.float32).T
             for w in (q_weight, k_weight, v_weight)], axis=1))
    aT = np.ascontiguousarray(
        np.concatenate(
            [np.asarray(a, dtype=np.float32).T for a in (q_A, k_A, v_A)],
            axis=1))
    bT = np.ascontiguousarray(
        np.stack([np.asarray(b, dtype=np.float32).T
                  for b in (q_B, k_B, v_B)]))

    in_maps = [
        {"xT": np.ascontiguousarray(xT[:, c * T:(c + 1) * T]),
         "wT": wT, "aT": aT, "bT": bT}
        for c in range(N_CORES)
    ]
    res = bass_utils.run_bass_kernel_spmd(
        nc, in_maps, core_ids=list(range(N_CORES)), trace=trace)

    full = []
    for name in ("q", "k", "v"):
        full.append(
            np.concatenate([res.results[c][name] for c in range(N_CORES)],
                           axis=0).reshape(Bb, S, H))
    return tuple(full), res


def kernel(**inputs):
    out, _ = _run(**inputs)
    return out



# revision 2
# speedup vs baseline: 1.1080x; 1.1080x over previous
"""LoRA QKV projection kernel for Trainium2 (Bass/Tile), 8-core SPMD.

Problem: x [B=4, S=2048, D=4096] fp32; for each of q/k/v:
    out = x @ W.T + (x @ A.T) @ B.T      (W [H=4096, D], A [R=16, D], B [H, R])

Sharding: data-parallel over tokens. Each of the 8 cores owns 1024 of the
8192 tokens and computes all 3*4096 output columns for them. Weights are
replicated.

On-device math runs the tensor engine in bf16 (both operands): measured
216 ns per 128x512 matmul vs 227 ns for f32r, and bf16 halves SBUF + HBM
traffic. End-to-end max rel err vs fp64 is ~2.4e-3 (tolerance 2e-2).

Layout notes:
- Host converts x/W/A/B to bf16 (round-to-nearest-even) and transposes so
  the contraction dim D lands on SBUF partitions; DMA goes straight into
  bf16 tiles (no on-chip casts).
- The three LoRA A matrices are fused into one [D, 48] operand so the
  xa = x @ A.T prologue is a single pass; the rank-16 closing matmul per
  chunk uses a [48, 512] B operand zero-padded outside its projection's
  16 rows, which keeps all stationary reads at partition base 0.
"""

import sys
import types

import numpy as np
import ml_dtypes

import concourse.bass as bass
import concourse.mybir as mybir
import concourse.tile as tile
from concourse import bacc, bass_utils


def _install_profiling_shim():
    """Make trace=True usable under axon on images whose ``antenv`` lacks
    ``axon_hooks``: inject the module and register the ctypes NTFF hook.
    Harmless no-op when the real module exists. Also keep profile artifacts
    local (no bucket upload is available here)."""
    try:
        if "antenv.axon_hooks" not in sys.modules:
            try:
                from antenv import axon_hooks  # noqa: F401
            except ImportError:
                mod = types.ModuleType("antenv.axon_hooks")
                mod._hook = None
                mod.set_axon_ntff_profile_hook = lambda h: setattr(
                    mod, "_hook", h)
                mod.get_axon_ntff_profile_hook = lambda: mod._hook
                sys.modules["antenv.axon_hooks"] = mod
                import antenv
                antenv.axon_hooks = mod
                try:
                    from trn_agent_boot.trn_boot import _ntff_profile_via_ctypes
                    hook = _ntff_profile_via_ctypes("/opt/axon/libaxon_pjrt.so")
                    if hook is not None:
                        mod.set_axon_ntff_profile_hook(hook)
                except Exception:
                    pass
        bass_utils.upload_artifacts = lambda tmpdir: "local://" + str(tmpdir)
    except Exception:
        pass


_install_profiling_shim()

F32 = mybir.dt.float32
BF16 = mybir.dt.bfloat16

N_CORES = 8
P = 128          # partition dim
CH = 512         # matmul moving free dim / psum bank width (fp32)
R3 = 48          # 3 stacked rank-16 LoRA blocks


def _build(D, T, H, n_cores=N_CORES):
    """Build the per-core Bass program.

    D: model dim (contraction), T: tokens per core, H: output columns per
    projection. All multiples of the tile sizes used below.
    """
    DT = D // P             # d-tiles
    ST = T // P             # token tiles per core (psum accumulators)
    CH_PER_PROJ = H // CH
    NCHUNK = 3 * CH_PER_PROJ
    SC = T // CH

    assert ST <= 8, "token tiles must fit in the 8 psum banks"

    nc = bacc.Bacc("TRN2", target_bir_lowering=False, debug=False,
                   num_devices=n_cores)

    xT_d = nc.dram_tensor("xT", [D, T], BF16, kind="ExternalInput")
    wT_d = nc.dram_tensor("wT", [D, 3 * H], BF16, kind="ExternalInput")
    a48_d = nc.dram_tensor("a48", [D, R3], BF16, kind="ExternalInput")
    b48_d = nc.dram_tensor("b48", [NCHUNK, R3, CH], BF16,
                           kind="ExternalInput")
    outs_d = [
        nc.dram_tensor(name, [T, H], F32, kind="ExternalOutput")
        for name in ("q", "k", "v")
    ]

    with tile.TileContext(nc) as tc:
        with (
            tc.tile_pool(name="xp", bufs=1) as xp,
            tc.tile_pool(name="wr", bufs=10) as wr,
            tc.tile_pool(name="br", bufs=3) as br,
            tc.tile_pool(name="psum", bufs=8, space="PSUM") as psum,
            tc.tile_pool(name="outsb", bufs=8) as outsb,
        ):
            # small LoRA-A operand first so it never queues behind x
            a48 = xp.tile([P, DT, R3], BF16, tag="a48")
            nc.sync.dma_start(
                a48[:], a48_d.rearrange("(dt p) r -> p dt r", p=P))

            xt = [xp.tile([P, T], BF16, tag="xt", bufs=DT, name=f"xt_{d}")
                  for d in range(DT)]
            for d in range(DT):
                nc.sync.dma_start(xt[d][:], xT_d[d * P:(d + 1) * P, :])

            # ---- prologue: xa[48, T] = (x @ A.T).T for all 3 projections,
            # DMA-paced inside the x-load window ----
            xa = xp.tile([R3, T], BF16, tag="xa")
            for sc in range(SC):
                pxa = psum.tile([R3, CH], F32, tag="ps", name=f"pxa_{sc}")
                for d in range(DT):
                    nc.tensor.matmul(
                        pxa[:],
                        a48[:, d, :],
                        xt[d][:, sc * CH:(sc + 1) * CH],
                        start=(d == 0),
                        stop=(d == DT - 1),
                    )
                nc.vector.tensor_copy(xa[:, sc * CH:(sc + 1) * CH], pxa[:])

            # ---- main loop: stream W.T chunks, accumulate in psum banks ----
            for j in range(NCHUNK):
                pj, hoff = j // CH_PER_PROJ, (j % CH_PER_PROJ) * CH
                b48 = br.tile([R3, CH], BF16, tag="b")
                nc.sync.dma_start(b48[:], b48_d[j])
                ps_tiles = [psum.tile([P, CH], F32, tag="ps",
                                      name=f"ps_{j}_{s}")
                            for s in range(ST)]
                for d in range(DT):
                    w = wr.tile([P, CH], BF16, tag="w")
                    nc.sync.dma_start(
                        w[:],
                        wT_d[d * P:(d + 1) * P,
                             pj * H + hoff:pj * H + hoff + CH],
                    )
                    for s in range(ST):
                        nc.tensor.matmul(
                            ps_tiles[s][:],
                            xt[d][:, s * P:(s + 1) * P],
                            w[:],
                            start=(d == 0),
                            stop=False,
                        )
                for s in range(ST):
                    # rank-48 (zero-padded) LoRA add closes the accumulation
                    nc.tensor.matmul(
                        ps_tiles[s][:],
                        xa[:, s * P:(s + 1) * P],
                        b48[:],
                        start=False,
                        stop=True,
                    )
                for s in range(ST):
                    ot = outsb.tile([P, CH], F32, tag="o")
                    nc.vector.tensor_copy(ot[:], ps_tiles[s][:])
                    nc.sync.dma_start(
                        outs_d[pj][s * P:(s + 1) * P, hoff:hoff + CH],
                        ot[:],
                    )

    nc.compile()
    return nc


_NC_CACHE = {}


def _get_nc(D, T, H):
    key = (D, T, H)
    if key not in _NC_CACHE:
        _NC_CACHE[key] = _build(D, T, H)
    return _NC_CACHE[key]


def _to_bf16(a):
    """f32 ndarray -> bf16 (round to nearest even), fast bit-twiddle."""
    a = np.ascontiguousarray(a, dtype=np.float32)
    u = a.view(np.uint32)
    rnd = (u >> 16) & 1
    b = ((u + np.uint32(0x7FFF) + rnd) >> 16).astype(np.uint16)
    return b.view(ml_dtypes.bfloat16)


def _run(x, q_weight, k_weight, v_weight, q_A, q_B, k_A, k_B, v_A, v_B,
         trace=False):
    Bb, S, D = x.shape
    H = q_weight.shape[0]
    TOK = Bb * S
    T = TOK // N_CORES
    CH_PER_PROJ = H // CH
    NCHUNK = 3 * CH_PER_PROJ

    nc = _get_nc(D, T, H)

    xT = _to_bf16(np.asarray(x, dtype=np.float32).reshape(TOK, D)).T
    wT = _to_bf16(
        np.concatenate(
            [np.asarray(w, dtype=np.float32).T
             for w in (q_weight, k_weight, v_weight)], axis=1))
    a48 = _to_bf16(
        np.concatenate(
            [np.asarray(a, dtype=np.float32).T for a in (q_A, k_A, v_A)],
            axis=1))
    b48 = np.zeros((NCHUNK, R3, CH), dtype=np.float32)
    for pj, Bm in enumerate((q_B, k_B, v_B)):
        BT = np.asarray(Bm, dtype=np.float32).T          # [16, H]
        for jj in range(CH_PER_PROJ):
            b48[pj * CH_PER_PROJ + jj, 16 * pj:16 * (pj + 1), :] = (
                BT[:, jj * CH:(jj + 1) * CH])
    b48 = _to_bf16(b48)

    in_maps = [
        {"xT": np.ascontiguousarray(xT[:, c * T:(c + 1) * T]),
         "wT": wT, "a48": a48, "b48": b48}
        for c in range(N_CORES)
    ]
    res = bass_utils.run_bass_kernel_spmd(
        nc, in_maps, core_ids=list(range(N_CORES)), trace=trace)

    full = []
    for name in ("q", "k", "v"):
        full.append(
            np.concatenate([res.results[c][name] for c in range(N_CORES)],
                           axis=0).reshape(Bb, S, H))
    return tuple(full), res


def kernel(**inputs):
    out, _ = _run(**inputs)
    return out
